# revision 11
# baseline (speedup 1.0000x reference)
"""BitLinear (ternary-weight + int8-activation quantized linear) on 8 Trainium2
NeuronCores, column-parallel over out_features.

Contract: kernel(x, weight) with x (2, 2048, 4096) f32, weight (16384, 4096) f32
returns (2, 2048, 16384) f32 — the full unsharded output.

Strategy
--------
- Shard weight rows (out_features) 8 ways; replicate x (the sharding hint).
- The quantized GEMM is exact integer math: |x_q| <= 127 fits bf16 exactly and
  the ternary weights {-1,0,+1} fit fp8e4m3 exactly, so a bf16(stationary) x
  fp8(moving) matmul with fp32 PSUM accumulation reproduces it bit-exactly;
  all scales fold into an fp32 epilogue (gamma * scale_w / 127 per token).
- scale_w = mean(|weight|) is a global reduction: pass 1 abs-sums each core's
  fp32 slice, a 4-byte AllReduce combines cores, pass 2 re-reads the slice,
  ternarizes in fp32 (round-half-even via the +/-(2^23+2^22) magic-add trick,
  matching jnp.round), converts bf16 -> DMA-xbar transpose -> fp8 into the
  SBUF-resident [d, o] weight. Pass 2 runs og-major so each 512-wide output
  group unblocks its matmuls as soon as its quarter of the weight is ready.
- x streams per 128-token tile: gamma = max|x| (fp32 reduce), ScalarE round
  magic -> bf16 -> xbar transpose to [d, tok]; TensorE accumulates 32 d-tiles
  per (token tile, output group); ScalarE applies the fp32 epilogue scale on
  the PSUM->SBUF copy. The first 4 token tiles iterate og-outer so the matmul
  phase starts while pass 2 is still ternarizing the later output groups.
"""

import sys

sys.path.insert(0, "/opt/trn_rl_repo")

import numpy as np

import concourse.bass as bass
import concourse.mybir as mybir
import concourse.tile as tile
import bass_rust
from concourse.bass_utils import run_bass_kernel_spmd

F32 = mybir.dt.float32
BF16 = mybir.dt.bfloat16
FP8 = mybir.dt.float8e4
CMAGIC = 12582912.0  # 2^23 + 2^22: (v + C) - C == round-half-even(v), |v| < 2^22
EPS = 1e-8

N_CORES = 8
B, T, D_IN, D_OUT = 2, 2048, 4096, 16384
TOK = B * T                      # 4096 tokens
OPC = D_OUT // N_CORES           # 2048 out features per core
NTOK = TOK // 128                # 32 token tiles
ND = D_IN // 128                 # 32 contraction tiles
NWC = OPC // 128                 # 16 weight row chunks per core
NOG = OPC // 512                 # 4 output groups per token tile
DH = D_IN // 2                   # 2048 staging width
NDH = DH // 128                  # 16 d-tiles per half
FIRST_BLOCK = 4                  # leading token tiles iterated og-outer


def _split_multi_waits(nc):
    """This container's walrus build rejects >1 sync wait per instruction, but
    Tile emits multi-wait instructions. Move extra waits onto preceding
    single-wait NoOps on the same engine (identical blocking semantics)."""
    wid = 0
    for f in nc.m.functions:
        for blk in f.blocks:
            insts = list(blk.instructions)
            new = []
            changed = False
            for inst in insts:
                si = inst.sync_info
                if si is not None and len(si.on_wait) > 1:
                    waits = list(si.on_wait)
                    for w in waits[:-1]:
                        nop = mybir.InstNoOp(name=f"WSPLIT-{wid}", ins=[], outs=[])
                        wid += 1
                        nop.engine = inst.engine
                        nop.sync_info = bass_rust.SyncInfo(on_wait=[w], on_update=[])
                        new.append(nop)
                    inst.sync_info = bass_rust.SyncInfo(
                        on_wait=[waits[-1]], on_update=list(si.on_update)
                    )
                    changed = True
                new.append(inst)
            if changed:
                blk.instructions = new


def build_bitlinear_nc():
    nc = bass.Bass("TRN2", target_bir_lowering=False, debug=False,
                   num_devices=N_CORES)
    x_d = nc.dram_tensor("x", [TOK, D_IN], F32, kind="ExternalInput")
    w_d = nc.dram_tensor("weight", [OPC, D_IN], F32, kind="ExternalInput")
    out_d = nc.dram_tensor("out", [TOK, OPC], F32, kind="ExternalOutput")
    cc_buf = nc.dram_tensor("cc_buf", [1, 1], F32)
    sc2_dram = nc.dram_tensor("sc2_d", [1, 2], F32)

    with tile.TileContext(nc, trace_sim=False) as tc:
        with (
            tc.tile_pool(name="wT", bufs=1) as wT_pool,
            tc.tile_pool(name="w32", bufs=4) as w32_pool,
            tc.tile_pool(name="x32", bufs=3) as x32_pool,
            tc.tile_pool(name="wt1", bufs=2) as wt1_pool,
            tc.tile_pool(name="xt1", bufs=2) as xt1_pool,
            tc.tile_pool(name="wtern", bufs=4) as wtern_pool,
            tc.tile_pool(name="wtT", bufs=2) as wtT_pool,
            tc.tile_pool(name="xq16", bufs=2) as xq16_pool,
            tc.tile_pool(name="xqT", bufs=FIRST_BLOCK) as xqT_pool,
            tc.tile_pool(name="outs", bufs=2) as outs_pool,
            tc.tile_pool(name="small", bufs=1) as small,
            tc.tile_pool(name="psum", bufs=2, space="PSUM") as psum_pool,
        ):
            # resident ternary weight, og-split: [d % 128, d // 128, o-in-group]
            w8 = [wT_pool.tile([128, ND, 512], FP8, tag=f"w8_{g}", name=f"w8_{g}")
                  for g in range(NOG)]
            partials = small.tile([128, 2 * NWC], F32)
            pass1_done = [None]
            cmag = small.tile([128, 1], F32)
            nc.gpsimd.memset(cmag[:], CMAGIC)

            # ---- pass 1: abs-sum of the fp32 weight slice ----
            for c in range(NWC):
                for h in range(2):
                    wchunk = w32_pool.tile([128, DH], F32, tag="w32")
                    nc.scalar.dma_start(
                        wchunk[:], w_d[c * 128:(c + 1) * 128, h * DH:(h + 1) * DH])
                    nc.vector.tensor_reduce(
                        partials[:, 2 * c + h:2 * c + h + 1], wchunk[:],
                        axis=mybir.AxisListType.X,
                        op=mybir.AluOpType.add, apply_absolute_value=True)

            # partials -> one scalar -> AllReduce across the 8 cores
            psum1 = small.tile([128, 1], F32)
            nc.vector.tensor_reduce(psum1[:], partials[:], axis=mybir.AxisListType.X,
                                    op=mybir.AluOpType.add)
            lsum = small.tile([1, 1], F32)
            p1inst = nc.gpsimd.tensor_reduce(lsum[:], psum1[:],
                                             axis=mybir.AxisListType.C,
                                             op=mybir.AluOpType.add)
            pass1_done[0] = p1inst.ins
            nc.scalar.dma_start(cc_buf[:], lsum[:])
            nc.gpsimd.collective_compute(
                "AllReduce", mybir.AluOpType.add,
                replica_groups=[list(range(N_CORES))],
                ins=[cc_buf[:]], outs=[cc_buf[:]])
            gsum = small.tile([1, 1], F32)
            nc.scalar.dma_start(gsum[:], cc_buf[:])

            # sc2 = [1/(scale_w + eps), scale_w/127], broadcast to 128 partitions
            sc2 = small.tile([1, 2], F32)
            nc.vector.tensor_scalar(sc2[:, 0:1], gsum[:], 1.0 / (D_OUT * D_IN), EPS,
                                    op0=mybir.AluOpType.mult,
                                    op1=mybir.AluOpType.add)
            nc.vector.reciprocal(sc2[:, 0:1], sc2[:, 0:1])
            nc.vector.tensor_scalar_mul(sc2[:, 1:2], gsum[:],
                                        1.0 / (D_OUT * D_IN * 127.0))
            nc.scalar.dma_start(sc2_dram[:], sc2[:])
            scb = small.tile([128, 2], F32)
            nc.scalar.dma_start(scb[:], sc2_dram[:].partition_broadcast(128))
            rsw_b = scb[:, 0:1]
            sw127_b = scb[:, 1:2]

            # ---- pass 2 (og-major): re-read fp32, ternarize, bf16, xbar, fp8 ----
            for c in range(NWC):
                g, cg = c // 4, c % 4
                for h in range(2):
                    wchunk = w32_pool.tile([128, DH], F32, tag="w32")
                    nc.scalar.dma_start(
                        wchunk[:], w_d[c * 128:(c + 1) * 128, h * DH:(h + 1) * DH])
                    tern = wtern_pool.tile([128, DH], BF16, tag="wtern")
                    for q in range(2):
                        sl = slice(q * 1024, (q + 1) * 1024)
                        tw = wt1_pool.tile([128, 1024], F32, tag="wt1")
                        nc.scalar.activation(tw[:], wchunk[:, sl],
                                             mybir.ActivationFunctionType.Identity,
                                             bias=cmag[:], scale=rsw_b)
                        nc.vector.tensor_scalar(tern[:, sl], tw[:], -CMAGIC, -1.0,
                                                op0=mybir.AluOpType.add,
                                                op1=mybir.AluOpType.max)
                        nc.vector.tensor_scalar_min(tern[:, sl], tern[:, sl], 1.0)
                    wtT = wtT_pool.tile([128, NDH, 128], BF16, tag="wtT")
                    # scalar HWDGE queue: keeps w transposes off the sync queue
                    # that carries the x transposes + out stores.
                    nc.scalar.dma_start_transpose(out=wtT[:], in_=tern[:])
                    # bf16 -> fp8 repack on VectorE: gpsimd takes ~7us per copy
                    # (serialized, it starved the PE for ~450us); DVE does it in ~1us.
                    nc.vector.tensor_copy(
                        w8[g][:, h * NDH:(h + 1) * NDH, cg * 128:(cg + 1) * 128],
                        wtT[:])

            # ---- x pipeline + matmuls ----
            xqTs = {}
            evecs = {}

            def x_tile_prep(t):
                xh = []
                gpart = small.tile([128, 2], F32, tag=f"gp{t % 4}",
                                   name=f"gp_{t}")
                for h in range(2):
                    xt = x32_pool.tile([128, DH], F32, tag="x32", name=f"x_{t}_{h}")
                    xdma = nc.sync.dma_start(
                        xt[:], x_d[t * 128:(t + 1) * 128, h * DH:(h + 1) * DH])
                    if t >= 2 and pass1_done[0] is not None:
                        tile.add_dep_helper(xdma.ins, pass1_done[0], sync=True,
                                            reason="x loads yield DMA BW to w pass-1")
                    nc.vector.tensor_reduce(gpart[:, h:h + 1], xt[:],
                                            axis=mybir.AxisListType.X,
                                            op=mybir.AluOpType.max,
                                            apply_absolute_value=True)
                    xh.append(xt)
                gv = small.tile([128, 2], F32, tag=f"gv{t % 4}", name=f"gv_{t}")
                gam, qs = gv[:, 0:1], gv[:, 1:2]
                nc.vector.tensor_reduce(gam, gpart[:], axis=mybir.AxisListType.X,
                                        op=mybir.AluOpType.max)
                nc.vector.tensor_scalar_add(qs, gam, EPS)
                nc.vector.reciprocal(qs, qs)
                nc.vector.tensor_scalar_mul(qs, qs, 127.0)
                evec = small.tile([128, 1], F32, tag=f"ev{t % 4}", name=f"ev_{t}")
                nc.vector.tensor_tensor(out=evec[:], in0=gam, in1=sw127_b,
                                        op=mybir.AluOpType.mult)
                evecs[t] = evec

                xqT = xqT_pool.tile([128, ND, 128], BF16, tag="xqT", name=f"xqT_{t}")
                for h in range(2):
                    xq16 = xq16_pool.tile([128, DH], BF16, tag="xq16")
                    for q in range(2):
                        sl = slice(q * 1024, (q + 1) * 1024)
                        x1 = xt1_pool.tile([128, 1024], F32, tag="xt1")
                        nc.scalar.activation(x1[:], xh[h][:, sl],
                                             mybir.ActivationFunctionType.Identity,
                                             bias=cmag[:], scale=qs)
                        nc.vector.tensor_scalar_add(xq16[:, sl], x1[:], -CMAGIC)
                    nc.sync.dma_start_transpose(
                        out=xqT[:, h * NDH:(h + 1) * NDH, :], in_=xq16[:])
                xqTs[t] = xqT

            def mm_group(t, og):
                acc = psum_pool.tile([128, 512], F32, tag=f"acc{og}",
                                     name=f"acc_{t}_{og}")
                xqT = xqTs[t]
                for k in range(ND):
                    nc.tensor.matmul(acc[:], xqT[:, k, :], w8[og][:, k, :],
                                     start=(k == 0), stop=(k == ND - 1))
                ot = outs_pool.tile([128, 512], F32, tag="outs")
                nc.scalar.activation(ot[:], acc[:],
                                     mybir.ActivationFunctionType.Copy,
                                     bias=0.0, scale=evecs[t][:])
                nc.sync.dma_start(
                    out_d[t * 128:(t + 1) * 128, og * 512:(og + 1) * 512], ot[:])

            # first block: og-outer so MMs start as soon as w8[0] is ready
            for t in range(FIRST_BLOCK):
                x_tile_prep(t)
            for og in range(NOG):
                for t in range(FIRST_BLOCK):
                    mm_group(t, og)
            # steady state: t-outer
            for t in range(FIRST_BLOCK, NTOK):
                x_tile_prep(t)
                for og in range(NOG):
                    mm_group(t, og)

    _split_multi_waits(nc)
    return nc


_NC_CACHE = None


def kernel(x: np.ndarray, weight: np.ndarray, _want_profile=False, **_kw):
    global _NC_CACHE
    assert x.shape == (B, T, D_IN) and weight.shape == (D_OUT, D_IN)
    x_flat = np.ascontiguousarray(x.reshape(TOK, D_IN), dtype=np.float32)
    w = np.ascontiguousarray(weight, dtype=np.float32)

    if _NC_CACHE is None:
        _NC_CACHE = build_bitlinear_nc()
    nc = _NC_CACHE

    in_maps = [
        {"x": x_flat, "weight": w[c * OPC:(c + 1) * OPC, :]}
        for c in range(N_CORES)
    ]
    res = run_bass_kernel_spmd(nc, in_maps, list(range(N_CORES)),
                               trace=bool(_want_profile))
    out = np.concatenate([res.results[c]["out"] for c in range(N_CORES)], axis=1)
    out = out.reshape(B, T, D_OUT)
    if _want_profile:
        return out, res
    return out



# revision 13
# speedup vs baseline: 1.4541x; 1.4541x over previous
"""BitLinear (ternary-weight + int8-activation quantized linear) on 8 Trainium2
NeuronCores, column-parallel over out_features.

Contract: kernel(x, weight) with x (2, 2048, 4096) f32, weight (16384, 4096) f32
returns (2, 2048, 16384) f32 — the full unsharded output.

Strategy
--------
- Shard weight rows (out_features) 8 ways; replicate x (the sharding hint).
- The quantized GEMM is exact integer math: |x_q| <= 127 fits bf16 exactly and
  the ternary weights {-1,0,+1} fit fp8e4m3 exactly, so a bf16(stationary) x
  fp8(moving) matmul with fp32 PSUM accumulation reproduces it bit-exactly;
  all scales fold into an fp32 epilogue (gamma * scale_w / 127 per token).
- scale_w = mean(|weight|) is a global reduction: pass 1 abs-sums each core's
  fp32 slice, a 4-byte AllReduce combines cores, pass 2 re-reads the slice,
  ternarizes in fp32 (round-half-even via the +/-(2^23+2^22) magic-add trick,
  matching jnp.round), converts bf16 -> DMA-xbar transpose -> fp8 into the
  SBUF-resident [d, o] weight. Pass 2 runs og-major so each 512-wide output
  group unblocks its matmuls as soon as its quarter of the weight is ready.
- x streams per 128-token tile: gamma = max|x| (fp32 reduce), ScalarE round
  magic -> bf16 -> xbar transpose to [d, tok]; TensorE accumulates 32 d-tiles
  per (token tile, output group); ScalarE applies the fp32 epilogue scale on
  the PSUM->SBUF copy. The first 4 token tiles iterate og-outer so the matmul
  phase starts while pass 2 is still ternarizing the later output groups.
"""

import sys

sys.path.insert(0, "/opt/trn_rl_repo")

import numpy as np

import concourse.bass as bass
import concourse.mybir as mybir
import concourse.tile as tile
import bass_rust
from concourse.bass_utils import run_bass_kernel_spmd

F32 = mybir.dt.float32
BF16 = mybir.dt.bfloat16
FP8 = mybir.dt.float8e4
CMAGIC = 12582912.0  # 2^23 + 2^22: (v + C) - C == round-half-even(v), |v| < 2^22
EPS = 1e-8

N_CORES = 8
B, T, D_IN, D_OUT = 2, 2048, 4096, 16384
TOK = B * T                      # 4096 tokens
OPC = D_OUT // N_CORES           # 2048 out features per core
NTOK = TOK // 128                # 32 token tiles
ND = D_IN // 128                 # 32 contraction tiles
NWC = OPC // 128                 # 16 weight row chunks per core
NOG = OPC // 512                 # 4 output groups per token tile
DH = D_IN // 2                   # 2048 staging width
NDH = DH // 128                  # 16 d-tiles per half
FIRST_BLOCK = 4                  # leading token tiles iterated og-outer


def _split_multi_waits(nc):
    """This container's walrus build rejects >1 sync wait per instruction, but
    Tile emits multi-wait instructions. Move extra waits onto preceding
    single-wait NoOps on the same engine (identical blocking semantics)."""
    wid = 0
    for f in nc.m.functions:
        for blk in f.blocks:
            insts = list(blk.instructions)
            new = []
            changed = False
            for inst in insts:
                si = inst.sync_info
                if si is not None and len(si.on_wait) > 1:
                    waits = list(si.on_wait)
                    for w in waits[:-1]:
                        nop = mybir.InstNoOp(name=f"WSPLIT-{wid}", ins=[], outs=[])
                        wid += 1
                        nop.engine = inst.engine
                        nop.sync_info = bass_rust.SyncInfo(on_wait=[w], on_update=[])
                        new.append(nop)
                    inst.sync_info = bass_rust.SyncInfo(
                        on_wait=[waits[-1]], on_update=list(si.on_update)
                    )
                    changed = True
                new.append(inst)
            if changed:
                blk.instructions = new


def build_bitlinear_nc():
    nc = bass.Bass("TRN2", target_bir_lowering=False, debug=False,
                   num_devices=N_CORES)
    x_d = nc.dram_tensor("x", [TOK, D_IN], F32, kind="ExternalInput")
    w_d = nc.dram_tensor("weight", [OPC, D_IN], F32, kind="ExternalInput")
    out_d = nc.dram_tensor("out", [TOK, OPC], F32, kind="ExternalOutput")
    cc_buf = nc.dram_tensor("cc_buf", [1, 1], F32)
    sc2_dram = nc.dram_tensor("sc2_d", [1, 2], F32)

    with tile.TileContext(nc, trace_sim=False) as tc:
        with (
            tc.tile_pool(name="wT", bufs=1) as wT_pool,
            tc.tile_pool(name="w32", bufs=4) as w32_pool,
            tc.tile_pool(name="x32", bufs=3) as x32_pool,
            tc.tile_pool(name="wt1", bufs=2) as wt1_pool,
            tc.tile_pool(name="xt1", bufs=2) as xt1_pool,
            tc.tile_pool(name="wtern", bufs=4) as wtern_pool,
            tc.tile_pool(name="wtT", bufs=2) as wtT_pool,
            tc.tile_pool(name="xq16", bufs=2) as xq16_pool,
            tc.tile_pool(name="xqT", bufs=FIRST_BLOCK) as xqT_pool,
            tc.tile_pool(name="outs", bufs=2) as outs_pool,
            tc.tile_pool(name="small", bufs=1) as small,
            tc.tile_pool(name="psum", bufs=2, space="PSUM") as psum_pool,
        ):
            # resident ternary weight, og-split: [d % 128, d // 128, o-in-group]
            w8 = [wT_pool.tile([128, ND, 512], FP8, tag=f"w8_{g}", name=f"w8_{g}")
                  for g in range(NOG)]
            partials = small.tile([128, 2 * NWC], F32)
            pass1_done = [None]
            cmag = small.tile([128, 1], F32)
            nc.gpsimd.memset(cmag[:], CMAGIC)

            # ---- pass 1: abs-sum of the fp32 weight slice ----
            for c in range(NWC):
                for h in range(2):
                    wchunk = w32_pool.tile([128, DH], F32, tag="w32")
                    nc.scalar.dma_start(
                        wchunk[:], w_d[c * 128:(c + 1) * 128, h * DH:(h + 1) * DH])
                    nc.vector.tensor_reduce(
                        partials[:, 2 * c + h:2 * c + h + 1], wchunk[:],
                        axis=mybir.AxisListType.X,
                        op=mybir.AluOpType.add, apply_absolute_value=True)

            # partials -> one scalar -> AllReduce across the 8 cores
            psum1 = small.tile([128, 1], F32)
            nc.vector.tensor_reduce(psum1[:], partials[:], axis=mybir.AxisListType.X,
                                    op=mybir.AluOpType.add)
            lsum = small.tile([1, 1], F32)
            p1inst = nc.gpsimd.tensor_reduce(lsum[:], psum1[:],
                                             axis=mybir.AxisListType.C,
                                             op=mybir.AluOpType.add)
            pass1_done[0] = p1inst.ins
            nc.scalar.dma_start(cc_buf[:], lsum[:])
            nc.gpsimd.collective_compute(
                "AllReduce", mybir.AluOpType.add,
                replica_groups=[list(range(N_CORES))],
                ins=[cc_buf[:]], outs=[cc_buf[:]])
            gsum = small.tile([1, 1], F32)
            nc.scalar.dma_start(gsum[:], cc_buf[:])

            # sc2 = [1/(scale_w + eps), scale_w/127], broadcast to 128 partitions
            sc2 = small.tile([1, 2], F32)
            nc.vector.tensor_scalar(sc2[:, 0:1], gsum[:], 1.0 / (D_OUT * D_IN), EPS,
                                    op0=mybir.AluOpType.mult,
                                    op1=mybir.AluOpType.add)
            nc.vector.reciprocal(sc2[:, 0:1], sc2[:, 0:1])
            nc.vector.tensor_scalar_mul(sc2[:, 1:2], gsum[:],
                                        1.0 / (D_OUT * D_IN * 127.0))
            nc.scalar.dma_start(sc2_dram[:], sc2[:])
            scb = small.tile([128, 2], F32)
            nc.scalar.dma_start(scb[:], sc2_dram[:].partition_broadcast(128))
            rsw_b = scb[:, 0:1]
            sw127_b = scb[:, 1:2]

            # ---- pass 2 (og-major): re-read fp32, ternarize, bf16, xbar, fp8 ----
            for c in range(NWC):
                g, cg = c // 4, c % 4
                for h in range(2):
                    wchunk = w32_pool.tile([128, DH], F32, tag="w32")
                    # SWDGE (gpsimd) issue: keeps the pass-2 re-read issues off
                    # the scalar engine stream, whose ternarize ACTs block on
                    # the AllReduce -- a blocked instruction stalls everything
                    # scheduled behind it on the same engine.
                    nc.gpsimd.dma_start(
                        wchunk[:], w_d[c * 128:(c + 1) * 128, h * DH:(h + 1) * DH])
                    tern = wtern_pool.tile([128, DH], BF16, tag="wtern")
                    for q in range(2):
                        sl = slice(q * 1024, (q + 1) * 1024)
                        tw = wt1_pool.tile([128, 1024], F32, tag="wt1")
                        nc.scalar.activation(tw[:], wchunk[:, sl],
                                             mybir.ActivationFunctionType.Identity,
                                             bias=cmag[:], scale=rsw_b)
                        nc.vector.tensor_scalar(tern[:, sl], tw[:], -CMAGIC, -1.0,
                                                op0=mybir.AluOpType.add,
                                                op1=mybir.AluOpType.max)
                        nc.vector.tensor_scalar_min(tern[:, sl], tern[:, sl], 1.0)
                    wtT = wtT_pool.tile([128, NDH, 128], BF16, tag="wtT")
                    nc.sync.dma_start_transpose(out=wtT[:], in_=tern[:])
                    # bf16 -> fp8 repack on VectorE: gpsimd takes ~7us per copy
                    # (serialized, it starved the PE for ~450us); DVE does it in ~1us.
                    nc.vector.tensor_copy(
                        w8[g][:, h * NDH:(h + 1) * NDH, cg * 128:(cg + 1) * 128],
                        wtT[:])

            # ---- x pipeline + matmuls ----
            xqTs = {}
            evecs = {}

            def x_tile_prep(t):
                xh = []
                gpart = small.tile([128, 2], F32, tag=f"gp{t % 4}",
                                   name=f"gp_{t}")
                for h in range(2):
                    xt = x32_pool.tile([128, DH], F32, tag="x32", name=f"x_{t}_{h}")
                    xdma = nc.sync.dma_start(
                        xt[:], x_d[t * 128:(t + 1) * 128, h * DH:(h + 1) * DH])
                    if t >= 2 and pass1_done[0] is not None:
                        tile.add_dep_helper(xdma.ins, pass1_done[0], sync=True,
                                            reason="x loads yield DMA BW to w pass-1")
                    nc.vector.tensor_reduce(gpart[:, h:h + 1], xt[:],
                                            axis=mybir.AxisListType.X,
                                            op=mybir.AluOpType.max,
                                            apply_absolute_value=True)
                    xh.append(xt)
                gv = small.tile([128, 2], F32, tag=f"gv{t % 4}", name=f"gv_{t}")
                gam, qs = gv[:, 0:1], gv[:, 1:2]
                nc.vector.tensor_reduce(gam, gpart[:], axis=mybir.AxisListType.X,
                                        op=mybir.AluOpType.max)
                nc.vector.tensor_scalar_add(qs, gam, EPS)
                nc.vector.reciprocal(qs, qs)
                nc.vector.tensor_scalar_mul(qs, qs, 127.0)
                evec = small.tile([128, 1], F32, tag=f"ev{t % 4}", name=f"ev_{t}")
                nc.vector.tensor_tensor(out=evec[:], in0=gam, in1=sw127_b,
                                        op=mybir.AluOpType.mult)
                evecs[t] = evec

                xqT = xqT_pool.tile([128, ND, 128], BF16, tag="xqT", name=f"xqT_{t}")
                for h in range(2):
                    xq16 = xq16_pool.tile([128, DH], BF16, tag="xq16")
                    for q in range(2):
                        sl = slice(q * 1024, (q + 1) * 1024)
                        x1 = xt1_pool.tile([128, 1024], F32, tag="xt1")
                        nc.scalar.activation(x1[:], xh[h][:, sl],
                                             mybir.ActivationFunctionType.Identity,
                                             bias=cmag[:], scale=qs)
                        nc.vector.tensor_scalar_add(xq16[:, sl], x1[:], -CMAGIC)
                    nc.sync.dma_start_transpose(
                        out=xqT[:, h * NDH:(h + 1) * NDH, :], in_=xq16[:])
                xqTs[t] = xqT

            def mm_group(t, og):
                acc = psum_pool.tile([128, 512], F32, tag=f"acc{og}",
                                     name=f"acc_{t}_{og}")
                xqT = xqTs[t]
                for k in range(ND):
                    nc.tensor.matmul(acc[:], xqT[:, k, :], w8[og][:, k, :],
                                     start=(k == 0), stop=(k == ND - 1))
                ot = outs_pool.tile([128, 512], F32, tag="outs")
                nc.scalar.activation(ot[:], acc[:],
                                     mybir.ActivationFunctionType.Copy,
                                     bias=0.0, scale=evecs[t][:])
                nc.sync.dma_start(
                    out_d[t * 128:(t + 1) * 128, og * 512:(og + 1) * 512], ot[:])

            # first block: og-outer so MMs start as soon as w8[0] is ready
            for t in range(FIRST_BLOCK):
                x_tile_prep(t)
            for og in range(NOG):
                for t in range(FIRST_BLOCK):
                    mm_group(t, og)
            # steady state: t-outer
            for t in range(FIRST_BLOCK, NTOK):
                x_tile_prep(t)
                for og in range(NOG):
                    mm_group(t, og)

    _split_multi_waits(nc)
    return nc


_NC_CACHE = None


def kernel(x: np.ndarray, weight: np.ndarray, _want_profile=False, **_kw):
    global _NC_CACHE
    assert x.shape == (B, T, D_IN) and weight.shape == (D_OUT, D_IN)
    x_flat = np.ascontiguousarray(x.reshape(TOK, D_IN), dtype=np.float32)
    w = np.ascontiguousarray(weight, dtype=np.float32)

    if _NC_CACHE is None:
        _NC_CACHE = build_bitlinear_nc()
    nc = _NC_CACHE

    in_maps = [
        {"x": x_flat, "weight": w[c * OPC:(c + 1) * OPC, :]}
        for c in range(N_CORES)
    ]
    res = run_bass_kernel_spmd(nc, in_maps, list(range(N_CORES)),
                               trace=bool(_want_profile))
    out = np.concatenate([res.results[c]["out"] for c in range(N_CORES)], axis=1)
    out = out.reshape(B, T, D_OUT)
    if _want_profile:
        return out, res
    return out

